# revision 12
# baseline (speedup 1.0000x reference)
"""Trainium2 Bass kernel for DecoderRNNCellJointCopy (B=16,T=600,V=20000,E=300,H=512).

Sharding across 8 NeuronCores:
  - LSTM stack: replicated (full batch, tiny work) on every core.
  - Bahdanau attentions: batch-parallel, 2 rows per core (row selection via a
    host-provided one-hot selector matrix so the program stays SPMD-uniform).
  - gen-prob Dense (Wg) + enc_ins copy-sum: vocab-parallel, V/8 = 2500 per core.
  - In-kernel AllGather shares per-row (exp copy-scores, hidden_att, S_c, switch);
    an AllReduce produces the global softmax denominator of the gen distribution.

Heavy matmuls run in float32r (full-rate single-pass fp32 mode, N>=256); small-N
matmuls stay plain fp32.
"""

import os

import numpy as np

import concourse.bass as bass
import concourse.bacc as bacc
import concourse.mybir as mybir
import concourse.tile as tile
from concourse.bass_utils import run_bass_kernel_spmd
from concourse.masks import make_identity

F32 = mybir.dt.float32
F32R = mybir.dt.float32r
AF = mybir.ActivationFunctionType

B, T, V, E, H = 16, 600, 20000, 300, 512
NC = 8
BL = B // NC          # batch rows per core (2)
VL = V // NC          # vocab slice per core (2500)
KE = E + H            # 812
KEP = 896             # padded to 7*128
NKE = KEP // 128      # 7
NH = H // 128         # 4
TCH = [128, 128, 128, 128, 88]   # T=600 partition chunks
NTC = len(TCH)
VCW = 500             # psum-bank-sized vocab chunk
NVC = VL // VCW       # 5
CB = 640 + H + 2      # per-row gather payload: exp scores (padded 640) + hidden + S_c + switch


def ts(i, w):
    return bass.ts(i, w)


def build_program():
    nc = bacc.Bacc("TRN2", target_bir_lowering=False, debug=False, num_devices=NC)

    def inp(name, shape):
        return nc.dram_tensor(name, list(shape), F32, kind="ExternalInput").ap()

    def outp(name, shape):
        return nc.dram_tensor(name, list(shape), F32, kind="ExternalOutput").ap()

    embT = inp("embT", (KEP, B))
    h1T = inp("h1T", (H, B))
    h2T = inp("h2T", (H, B))
    c1_in = inp("c1_in", (B, H))
    c2_in = inp("c2_in", (B, H))
    k1p = inp("k1p", (KEP, 4 * H))
    r1w = inp("r1w", (H, 4 * H))
    k2w = inp("k2w", (H, 4 * H))
    r2w = inp("r2w", (H, 4 * H))
    b1v = inp("b1v", (1, 4 * H))
    b2v = inp("b2v", (1, 4 * H))
    aw1g = inp("aw1g", (H, H))
    aw2g = inp("aw2g", (H, H))
    avg = inp("avg", (H, 1))
    aw1c = inp("aw1c", (H, H))
    aw2c = inp("aw2c", (H, H))
    avc = inp("avc", (H, 1))
    whw = inp("whw", (2 * H, H))
    bhT = inp("bhT", (H, 1))
    wfw = inp("wfw", (H, 1))
    bfv = inp("bfv", (1, 1))
    sel = inp("sel", (B, BL))
    enc_outs_loc = inp("enc_outs_loc", (BL, T, H))
    wg_loc = inp("wg_loc", (H, VL))
    bg_loc = inp("bg_loc", (1, VL))
    enc_ins_loc = inp("enc_ins_loc", (B, T, VL))

    out_result = outp("result_loc", (B, VL))
    out_hidden = outp("hidden_att", (B, H))
    out_h1n = outp("h1n", (B, H))
    out_c1n = outp("c1n", (B, H))
    out_h2n = outp("h2n", (B, H))
    out_c2n = outp("c2n", (B, H))

    with tile.TileContext(nc) as tc:
        _body(nc, tc, locals())
    nc.compile()
    return nc


def _body(nc, tc, io):
    embT, h1T, h2T = io["embT"], io["h1T"], io["h2T"]
    c1_in, c2_in = io["c1_in"], io["c2_in"]
    k1p, r1w, k2w, r2w, b1v, b2v = (
        io["k1p"], io["r1w"], io["k2w"], io["r2w"], io["b1v"], io["b2v"])
    aw1g, aw2g, avg = io["aw1g"], io["aw2g"], io["avg"]
    aw1c, aw2c, avc = io["aw1c"], io["aw2c"], io["avc"]
    whw, bhT, wfw, bfv, sel = io["whw"], io["bhT"], io["wfw"], io["bfv"], io["sel"]
    enc_outs_loc, wg_loc, bg_loc, enc_ins_loc = (
        io["enc_outs_loc"], io["wg_loc"], io["bg_loc"], io["enc_ins_loc"])
    out_result, out_hidden = io["out_result"], io["out_hidden"]
    out_h1n, out_c1n, out_h2n, out_c2n = (
        io["out_h1n"], io["out_c1n"], io["out_h2n"], io["out_c2n"])

    with (
        tc.tile_pool(name="pers", bufs=1) as pers,
        tc.tile_pool(name="enc", bufs=3) as encp,
        tc.tile_pool(name="dram", bufs=1, space="DRAM") as dram,
    ):
        # ---------- persistent small tiles ----------
        ones = pers.tile([1, 128], F32)
        nc.gpsimd.memset(ones[:], 1.0)
        ones_r = pers.tile([1, 128], F32R)
        nc.vector.tensor_copy(ones_r[:], ones[:])
        onesC = pers.tile([128, 1], F32)
        nc.gpsimd.memset(onesC[:], 1.0)
        ident = pers.tile([128, 128], F32)
        make_identity(nc, ident[:])

        embT_sb = pers.tile([128, NKE, B], F32R)
        nc.sync.dma_start(embT_sb[:], embT.rearrange("(c p) n -> p c n", p=128).bitcast(F32R))
        h1T_sb = pers.tile([128, NH, B], F32R)
        nc.sync.dma_start(h1T_sb[:], h1T.rearrange("(c p) n -> p c n", p=128).bitcast(F32R))
        h2T_sb = pers.tile([128, NH, B], F32R)
        nc.sync.dma_start(h2T_sb[:], h2T.rearrange("(c p) n -> p c n", p=128).bitcast(F32R))
        c1_sb = pers.tile([B, H], F32)
        nc.sync.dma_start(c1_sb[:], c1_in)
        c2_sb = pers.tile([B, H], F32)
        nc.sync.dma_start(c2_sb[:], c2_in)
        b1_sb = pers.tile([1, 4 * H], F32R)
        nc.sync.dma_start(b1_sb[:], b1v.bitcast(F32R))
        b2_sb = pers.tile([1, 4 * H], F32R)
        nc.sync.dma_start(b2_sb[:], b2v.bitcast(F32R))
        sel_sb = pers.tile([B, BL], F32)
        nc.sync.dma_start(sel_sb[:], sel)

        # attention weights up-front (small, off the critical path later)
        def _wload(apin, c, n, dt=F32):
            t = pers.tile([128, c, n], dt, tag=f"w_{apin.tensor.name}")
            src = apin.rearrange("(c p) n -> p c n", p=128)
            if dt == F32R:
                src = src.bitcast(F32R)
            nc.sync.dma_start(t[:], src)
            return t

        aw1g_sb = _wload(aw1g, NH, H)
        aw2g_sb = _wload(aw2g, NH, H, F32R)
        aw1c_sb = _wload(aw1c, NH, H)
        aw2c_sb = _wload(aw2c, NH, H, F32R)
        wh_sb = _wload(whw, 2 * NH, H)
        avg_sb = pers.tile([128, NH], F32)
        nc.sync.dma_start(avg_sb[:], avg.rearrange("(c p) 1 -> p c", p=128))
        avc_sb = pers.tile([128, NH], F32)
        nc.sync.dma_start(avc_sb[:], avc.rearrange("(c p) 1 -> p c", p=128))
        wf_sb = pers.tile([128, NH], F32)
        nc.sync.dma_start(wf_sb[:], wfw.rearrange("(c p) 1 -> p c", p=128))
        bhT_sb = pers.tile([128, NH], F32)
        nc.sync.dma_start(bhT_sb[:], bhT.rearrange("(c p) 1 -> p c", p=128))
        bf_sb = pers.tile([1, 1], F32)
        nc.sync.dma_start(bf_sb[:], bfv)

        # persistent state tiles
        h1n_sb = pers.tile([B, H], F32)
        c1n_sb = pers.tile([B, H], F32)
        h2n_sb = pers.tile([B, H], F32)
        c2n_sb = pers.tile([B, H], F32)
        h1nT_sb = pers.tile([128, NH, B], F32R)
        h2nTl_sb = pers.tile([128, NH, BL], F32)
        qg_sb = pers.tile([128, NH, BL], F32)
        qc_sb = pers.tile([128, NH, BL], F32)
        escg_sb = pers.tile([128, BL, NTC], F32)
        escc_sb = pers.tile([128, BL, NTC], F32)
        catT_sb = pers.tile([128, 2 * NH, BL], F32)
        hattT_sb = pers.tile([128, NH, BL], F32)
        switch_sb = pers.tile([BL, 1], F32)
        sc_sb = pers.tile([1, BL], F32)      # raw copy-attention denominators
        rsg_sb = pers.tile([1, BL], F32)     # 1/S of gen-attention softmax
        rsbc_sb = pers.tile([128, BL], F32)  # broadcast of rsg over partitions
        # consumer-side (all-batch) tiles
        escT_sb = pers.tile([128, NTC, B], F32R)
        hattT_all = pers.tile([128, NH, B], F32R)
        swall_sb = pers.tile([B, 1], F32)
        scall_sb = pers.tile([B, 1], F32)
        sgpart_sb = pers.tile([B, NVC], F32)
        sgsum_sb = pers.tile([B, 1], F32)
        recipg_sb = pers.tile([B, 1], F32)
        recipc_sb = pers.tile([B, 1], F32)
        amix_sb = pers.tile([B, 1], F32)
        cmix_sb = pers.tile([B, 1], F32)

        # DRAM bounce buffers for collectives
        cbuf = dram.tile([BL, CB], F32)
        gath = dram.tile([B, CB], F32)
        sgin = dram.tile([B], F32)
        sgout = dram.tile([B], F32)

        # ---------- enc_ins stream (starts immediately; gated only by pool slots) ----------
        # allocated lazily in the copy phase below via encp.tile(tag="enc")

        # ---------- phase A: LSTM stack ----------
        with (
            tc.tile_pool(name="w1", bufs=1) as w1p,
            tc.tile_pool(name="psA", bufs=4, space="PSUM") as psA,
            tc.tile_pool(name="psTPa", bufs=2, space="PSUM") as psTPa,
            tc.tile_pool(name="gts", bufs=1) as gts,
        ):
            k1_sb = w1p.tile([128, NKE, 4 * H], F32R)
            nc.sync.dma_start(k1_sb[:], k1p.rearrange("(c p) n -> p c n", p=128).bitcast(F32R))
            r1_sb = w1p.tile([128, NH, 4 * H], F32R)
            nc.sync.dma_start(r1_sb[:], r1w.rearrange("(c p) n -> p c n", p=128).bitcast(F32R))

            def z_gates(xT_sb, nx, kw_sb, hT_sb, rw_sb, bias_sb, c_sb, hn_sb, cn_sb):
                zps = []
                for j in range(4):
                    ps = psA.tile([B, H], F32, tag="z")
                    for kc in range(nx):
                        nc.tensor.matmul(ps[:], xT_sb[:, kc, :],
                                         kw_sb[:, kc, ts(j, H)],
                                         start=(kc == 0), stop=False)
                    for kc in range(NH):
                        nc.tensor.matmul(ps[:], hT_sb[:, kc, :],
                                         rw_sb[:, kc, ts(j, H)],
                                         start=False, stop=False)
                    nc.tensor.matmul(ps[:], ones_r[0:1, 0:B],
                                     bias_sb[0:1, ts(j, H)],
                                     start=False, stop=True)
                    zps.append(ps)
                i_s = gts.tile([B, H], F32, tag="i_s")
                f_s = gts.tile([B, H], F32, tag="f_s")
                g_t = gts.tile([B, H], F32, tag="g_t")
                o_s = gts.tile([B, H], F32, tag="o_s")
                nc.scalar.activation(i_s[:], zps[0][:], AF.Sigmoid)
                nc.scalar.activation(f_s[:], zps[1][:], AF.Sigmoid)
                nc.scalar.activation(g_t[:], zps[2][:], AF.Tanh)
                nc.scalar.activation(o_s[:], zps[3][:], AF.Sigmoid)
                nc.vector.tensor_mul(f_s[:], f_s[:], c_sb[:])
                nc.vector.tensor_mul(i_s[:], i_s[:], g_t[:])
                nc.vector.tensor_add(cn_sb[:], f_s[:], i_s[:])
                nc.scalar.activation(g_t[:], cn_sb[:], AF.Tanh)
                nc.vector.tensor_mul(hn_sb[:], o_s[:], g_t[:])

            z_gates(embT_sb, NKE, k1_sb, h1T_sb, r1_sb, b1_sb, c1_sb, h1n_sb, c1n_sb)
            # transpose h1n -> [H, B] chunks for LSTM2 input
            for m in range(NH):
                pst = psTPa.tile([128, B], F32, tag="tpa")
                nc.tensor.transpose(pst[:], h1n_sb[:, ts(m, 128)], ident[0:B, 0:B])
                nc.scalar.activation(h1nT_sb[:, m, :], pst[:], AF.Copy)

        with (
            tc.tile_pool(name="w2", bufs=1) as w2p,
            tc.tile_pool(name="psA2", bufs=4, space="PSUM") as psA2,
            tc.tile_pool(name="gts2", bufs=1) as gts2,
        ):
            k2_sb = w2p.tile([128, NH, 4 * H], F32R)
            nc.sync.dma_start(k2_sb[:], k2w.rearrange("(c p) n -> p c n", p=128).bitcast(F32R))
            r2_sb = w2p.tile([128, NH, 4 * H], F32R)
            nc.sync.dma_start(r2_sb[:], r2w.rearrange("(c p) n -> p c n", p=128).bitcast(F32R))
            zps = []
            for j in range(4):
                ps = psA2.tile([B, H], F32, tag="z2")
                for kc in range(NH):
                    nc.tensor.matmul(ps[:], h1nT_sb[:, kc, :],
                                     k2_sb[:, kc, ts(j, H)],
                                     start=(kc == 0), stop=False)
                for kc in range(NH):
                    nc.tensor.matmul(ps[:], h2T_sb[:, kc, :],
                                     r2_sb[:, kc, ts(j, H)],
                                     start=False, stop=False)
                nc.tensor.matmul(ps[:], ones_r[0:1, 0:B],
                                 b2_sb[0:1, ts(j, H)],
                                 start=False, stop=True)
                zps.append(ps)
            i_s = gts2.tile([B, H], F32, tag="i_s2")
            f_s = gts2.tile([B, H], F32, tag="f_s2")
            g_t = gts2.tile([B, H], F32, tag="g_t2")
            o_s = gts2.tile([B, H], F32, tag="o_s2")
            nc.scalar.activation(i_s[:], zps[0][:], AF.Sigmoid)
            nc.scalar.activation(f_s[:], zps[1][:], AF.Sigmoid)
            nc.scalar.activation(g_t[:], zps[2][:], AF.Tanh)
            nc.scalar.activation(o_s[:], zps[3][:], AF.Sigmoid)
            nc.vector.tensor_mul(f_s[:], f_s[:], c2_sb[:])
            nc.vector.tensor_mul(i_s[:], i_s[:], g_t[:])
            nc.vector.tensor_add(c2n_sb[:], f_s[:], i_s[:])
            nc.scalar.activation(g_t[:], c2n_sb[:], AF.Tanh)
            nc.vector.tensor_mul(h2n_sb[:], o_s[:], g_t[:])

        # state outputs ready
        nc.gpsimd.dma_start(out_h1n, h1n_sb[:])
        nc.gpsimd.dma_start(out_c1n, c1n_sb[:])
        nc.gpsimd.dma_start(out_h2n, h2n_sb[:])
        nc.gpsimd.dma_start(out_c2n, c2n_sb[:])

        # ---------- phase B: attentions (local rows) ----------
        with (
            tc.tile_pool(name="attn", bufs=1) as attp,
            tc.tile_pool(name="tv", bufs=4) as tvp,
            tc.tile_pool(name="psS1", bufs=2, space="PSUM") as psS1,
            tc.tile_pool(name="psSC", bufs=2, space="PSUM") as psSC,
            tc.tile_pool(name="psM", bufs=3, space="PSUM") as psM,
        ):
            X_sb = attp.tile([128, BL, NTC, H], F32)
            for lb in range(BL):
                for tci, tw in enumerate(TCH):
                    nc.sync.dma_start(X_sb[0:tw, lb, tci, :],
                                      enc_outs_loc[lb, tci * 128:tci * 128 + tw, :])
            # local h2n columns via selector matmul: h2nT_loc = h2n^T @ sel
            for m in range(NH):
                psq = psM.tile([128, BL], F32, tag="m")
                nc.tensor.matmul(psq[:], h2n_sb[:, ts(m, 128)], sel_sb[:],
                                 start=True, stop=True)
                nc.scalar.activation(h2nTl_sb[:, m, :], psq[:], AF.Copy)
            # XT chunks
            XT_sb = attp.tile([128, BL, NH, T], F32R)
            for lb in range(BL):
                for tci, tw in enumerate(TCH):
                    for m in range(NH):
                        pst = psM.tile([128, 128], F32, tag="m")
                        nc.tensor.transpose(pst[0:128, 0:tw],
                                            X_sb[0:tw, lb, tci, ts(m, 128)],
                                            ident[0:tw, 0:tw])
                        nc.scalar.activation(
                            XT_sb[:, lb, m, tci * 128:tci * 128 + tw],
                            pst[0:128, 0:tw], AF.Copy)

            for w1_sb, w2_sb, av_sb, q_sb, esc_sb, is_gen in (
                (aw1g_sb, aw2g_sb, avg_sb, qg_sb, escg_sb, True),
                (aw1c_sb, aw2c_sb, avc_sb, qc_sb, escc_sb, False),
            ):
                # query projection qT = w1^T @ h2nT_loc
                for m in range(NH):
                    psq = psM.tile([128, BL], F32, tag="m")
                    for kc in range(NH):
                        nc.tensor.matmul(psq[:], w1_sb[:, kc, ts(m, 128)],
                                         h2nTl_sb[:, kc, :],
                                         start=(kc == 0), stop=(kc == NH - 1))
                    nc.scalar.activation(q_sb[:, m, :], psq[:], AF.Copy)
                for lb in range(BL):
                    nc.gpsimd.memset(esc_sb[:, lb, :], 0.0)
                    tvs = []
                    for m in range(NH):
                        tv = tvp.tile([128, T], F32, tag="tv")
                        for t0, twd in ((0, 512), (512, 88)):
                            ps1 = psS1.tile([128, 512], F32, tag="s1")
                            for kc in range(NH):
                                nc.tensor.matmul(
                                    ps1[:, 0:twd],
                                    w2_sb[:, kc, ts(m, 128)],
                                    XT_sb[:, lb, kc, t0:t0 + twd],
                                    start=(kc == 0), stop=(kc == NH - 1))
                            nc.scalar.activation(tv[:, t0:t0 + twd], ps1[:, 0:twd],
                                                 AF.Tanh, bias=q_sb[:, m, lb:lb + 1])
                        tvs.append(tv)
                    # scores (t-partitioned), exp
                    for tci, tw in enumerate(TCH):
                        pssc = psSC.tile([128, 1], F32, tag="sc")
                        for m in range(NH):
                            nc.tensor.matmul(
                                pssc[0:tw, 0:1],
                                tvs[m][:, tci * 128:tci * 128 + tw],
                                av_sb[:, m:m + 1],
                                start=(m == 0), stop=(m == NH - 1))
                        nc.scalar.activation(esc_sb[0:tw, lb, tci:tci + 1],
                                             pssc[0:tw, 0:1], AF.Exp)
                    # denominator
                    psS = psM.tile([128, 1], F32, tag="m")
                    for tci, tw in enumerate(TCH):
                        nc.tensor.matmul(psS[0:1, 0:1],
                                         esc_sb[0:tw, lb, tci:tci + 1],
                                         onesC[0:tw, 0:1],
                                         start=(tci == 0), stop=(tci == NTC - 1))
                    if is_gen:
                        nc.vector.reciprocal(rsg_sb[0:1, lb:lb + 1], psS[0:1, 0:1])
                        psb = psM.tile([128, 1], F32, tag="m")
                        nc.tensor.matmul(psb[:], ones[0:1, :],
                                         rsg_sb[0:1, lb:lb + 1],
                                         start=True, stop=True)
                        nc.scalar.activation(rsbc_sb[:, lb:lb + 1], psb[:], AF.Copy)
                        # context (normalized) into catT rows 0..511
                        for m in range(NH):
                            psc = psM.tile([128, 1], F32, tag="m")
                            for tci, tw in enumerate(TCH):
                                nc.tensor.matmul(psc[:],
                                                 X_sb[0:tw, lb, tci, ts(m, 128)],
                                                 esc_sb[0:tw, lb, tci:tci + 1],
                                                 start=(tci == 0),
                                                 stop=(tci == NTC - 1))
                            nc.vector.tensor_scalar_mul(catT_sb[:, m, lb:lb + 1],
                                                        psc[:], rsbc_sb[:, lb:lb + 1])
                    else:
                        nc.scalar.activation(sc_sb[0:1, lb:lb + 1], psS[0:1, 0:1],
                                             AF.Copy)

            # catT rows 512..1023 = h2n_loc
            for m in range(NH):
                nc.vector.tensor_copy(catT_sb[:, NH + m, :], h2nTl_sb[:, m, :])
            # hidden_att (transposed, local cols)
            for m in range(NH):
                psh = psM.tile([128, BL], F32, tag="m")
                for kc in range(2 * NH):
                    nc.tensor.matmul(psh[:], wh_sb[:, kc, ts(m, 128)],
                                     catT_sb[:, kc, :],
                                     start=(kc == 0), stop=(kc == 2 * NH - 1))
                nc.scalar.activation(hattT_sb[:, m, :], psh[:], AF.Tanh,
                                     bias=bhT_sb[:, m:m + 1])
            # switch
            pss = psM.tile([128, 1], F32, tag="m")
            for m in range(NH):
                nc.tensor.matmul(pss[0:BL, 0:1], hattT_sb[:, m, :],
                                 wf_sb[:, m:m + 1],
                                 start=(m == 0), stop=False)
            nc.tensor.matmul(pss[0:BL, 0:1], ones[0:1, 0:BL], bf_sb[0:1, 0:1],
                             start=False, stop=True)
            nc.scalar.activation(switch_sb[:], pss[0:BL, 0:1], AF.Sigmoid)

            # contribution buffer writes
            for lb in range(BL):
                for tci in range(NTC):
                    nc.gpsimd.dma_start(cbuf[lb, tci * 128:(tci + 1) * 128],
                                        escc_sb[:, lb, tci])
                for m in range(NH):
                    nc.gpsimd.dma_start(
                        cbuf[lb, 640 + m * 128:640 + (m + 1) * 128],
                        hattT_sb[:, m, lb])
                nc.gpsimd.dma_start(cbuf[lb, 640 + H:640 + H + 1],
                                    sc_sb[0:1, lb])
                nc.gpsimd.dma_start(cbuf[lb, 640 + H + 1:640 + H + 2],
                                    switch_sb[lb, 0:1])

        # ---------- AllGather ----------
        if os.environ.get("K_NO_COLLECTIVE"):
            nc.gpsimd.dma_start(gath[0:BL, :], cbuf[:, :])
        else:
            nc.gpsimd.collective_compute(
                "AllGather", mybir.AluOpType.bypass,
                replica_groups=[list(range(NC))],
                ins=[cbuf.opt()], outs=[gath.opt()])

        # ---------- phase C: gen dense + copy-sum on vocab slice ----------
        with (
            tc.tile_pool(name="gen", bufs=1) as genp,
            tc.tile_pool(name="stg", bufs=1) as stgp,
            tc.tile_pool(name="psTPc", bufs=1, space="PSUM") as psTPc,
            tc.tile_pool(name="psG", bufs=2, space="PSUM") as psG,
            tc.tile_pool(name="psCp", bufs=5, space="PSUM") as psCp,
        ):
            wg_sb = genp.tile([128, NH, VL], F32R)
            nc.sync.dma_start(wg_sb[:], wg_loc.rearrange("(c p) n -> p c n", p=128).bitcast(F32R))
            bg_sb = genp.tile([1, VL], F32R)
            nc.sync.dma_start(bg_sb[:], bg_loc.bitcast(F32R))
            escore_rows = genp.tile([B, 640], F32)
            nc.gpsimd.dma_start(escore_rows[:], gath[:, 0:640])
            hidden_rows = genp.tile([B, H], F32)
            nc.gpsimd.dma_start(hidden_rows[:], gath[:, 640:640 + H])
            nc.gpsimd.dma_start(scall_sb[:], gath[:, 640 + H:640 + H + 1])
            nc.gpsimd.dma_start(swall_sb[:], gath[:, 640 + H + 1:640 + H + 2])
            nc.gpsimd.dma_start(out_hidden, gath[:, 640:640 + H])

            for tci in range(NTC):
                pst = psTPc.tile([128, B], F32, tag="tpc")
                nc.tensor.transpose(pst[:], escore_rows[:, ts(tci, 128)],
                                    ident[0:B, 0:B])
                nc.scalar.activation(escT_sb[:, tci, :], pst[:], AF.Copy)
            for m in range(NH):
                pst = psTPc.tile([128, B], F32, tag="tpc")
                nc.tensor.transpose(pst[:], hidden_rows[:, ts(m, 128)],
                                    ident[0:B, 0:B])
                nc.scalar.activation(hattT_all[:, m, :], pst[:], AF.Copy)

            genexp_sb = genp.tile([B, VL], F32)
            for vc in range(NVC):
                pg = psG.tile([B, 512], F32, tag="pg")
                for kc in range(NH):
                    nc.tensor.matmul(pg[:, 0:VCW], hattT_all[:, kc, :],
                                     wg_sb[:, kc, ts(vc, VCW)],
                                     start=(kc == 0), stop=False)
                nc.tensor.matmul(pg[:, 0:VCW], ones_r[0:1, 0:B],
                                 bg_sb[0:1, ts(vc, VCW)],
                                 start=False, stop=True)
                nc.scalar.activation(genexp_sb[:, ts(vc, VCW)], pg[:, 0:VCW],
                                     AF.Exp, accum_out=sgpart_sb[:, vc:vc + 1])
            nc.vector.tensor_reduce(sgsum_sb[:], sgpart_sb[:],
                                    axis=mybir.AxisListType.X,
                                    op=mybir.AluOpType.add)
            nc.gpsimd.dma_start(sgin[:], sgsum_sb[:])
            if os.environ.get("K_NO_COLLECTIVE"):
                nc.gpsimd.dma_start(sgout[:], sgin[:])
            else:
                nc.gpsimd.collective_compute(
                    "AllReduce", mybir.AluOpType.add,
                    replica_groups=[list(range(NC))],
                    ins=[sgin.opt()], outs=[sgout.opt()])
            nc.gpsimd.dma_start(sgsum_sb[:], sgout[:])
            nc.vector.reciprocal(recipg_sb[:], sgsum_sb[:])
            nc.vector.reciprocal(recipc_sb[:], scall_sb[:])
            nc.vector.tensor_scalar(amix_sb[:], swall_sb[:], -1.0, 1.0,
                                    op0=mybir.AluOpType.mult,
                                    op1=mybir.AluOpType.add)
            nc.vector.tensor_mul(amix_sb[:], amix_sb[:], recipg_sb[:])
            nc.vector.tensor_mul(cmix_sb[:], swall_sb[:], recipc_sb[:])

            # copy-sum over the enc_ins stream
            copyfull_sb = genp.tile([B, VL], F32)
            for g in range(B):
                pcs = [psCp.tile([1, 512], F32, tag="cp", name=f"cp{vc}")
                       for vc in range(NVC)]
                for tci, tw in enumerate(TCH):
                    et = encp.tile([128, VL], F32R, tag="enc")
                    nc.sync.dma_start(et[0:tw, :],
                                      enc_ins_loc[g, tci * 128:tci * 128 + tw, :].bitcast(F32R))
                    for vc in range(NVC):
                        nc.tensor.matmul(pcs[vc][0:1, 0:VCW],
                                         escT_sb[0:tw, tci, g:g + 1],
                                         et[0:tw, ts(vc, VCW)],
                                         start=(tci == 0), stop=(tci == NTC - 1))
                stage = stgp.tile([1, VL], F32, tag="stage")
                for vc in range(NVC):
                    nc.scalar.activation(stage[0:1, ts(vc, VCW)],
                                         pcs[vc][0:1, 0:VCW], AF.Copy)
                nc.gpsimd.dma_start(copyfull_sb[g:g + 1, :], stage[0:1, :])

            # final mix and output
            fin_sb = genp.tile([B, VL], F32)
            for vc in range(NVC):
                sl = ts(vc, VCW)
                nc.vector.tensor_scalar_mul(fin_sb[:, sl], genexp_sb[:, sl],
                                            amix_sb[:])
                nc.vector.tensor_scalar_mul(copyfull_sb[:, sl], copyfull_sb[:, sl],
                                            cmix_sb[:])
                nc.vector.tensor_add(fin_sb[:, sl], fin_sb[:, sl],
                                     copyfull_sb[:, sl])
            nc.sync.dma_start(out_result, fin_sb[:])


_PROGRAM = None


def _get_program():
    global _PROGRAM
    if _PROGRAM is None:
        _PROGRAM = build_program()
    return _PROGRAM


def make_in_maps(inputs):
    x = np.asarray(inputs["x"])
    emb_table = np.asarray(inputs["emb_table"], np.float32)
    emb = emb_table[x[:, 0]]                                   # [B, E]
    emb_att = np.concatenate(
        [emb, np.asarray(inputs["last_hidden_attn"], np.float32)], axis=1)
    embT = np.zeros((KEP, B), np.float32)
    embT[:KE, :] = emb_att.T
    k1 = np.asarray(inputs["k1"], np.float32)
    k1p = np.zeros((KEP, 4 * H), np.float32)
    k1p[:KE, :] = k1

    common = {
        "embT": np.ascontiguousarray(embT),
        "h1T": np.ascontiguousarray(np.asarray(inputs["h1"], np.float32).T),
        "h2T": np.ascontiguousarray(np.asarray(inputs["h2"], np.float32).T),
        "c1_in": np.ascontiguousarray(inputs["c1"], dtype=np.float32),
        "c2_in": np.ascontiguousarray(inputs["c2"], dtype=np.float32),
        "k1p": k1p,
        "r1w": np.ascontiguousarray(inputs["r1"], dtype=np.float32),
        "k2w": np.ascontiguousarray(inputs["k2"], dtype=np.float32),
        "r2w": np.ascontiguousarray(inputs["r2"], dtype=np.float32),
        "b1v": np.ascontiguousarray(inputs["b1"], dtype=np.float32).reshape(1, 4 * H),
        "b2v": np.ascontiguousarray(inputs["b2"], dtype=np.float32).reshape(1, 4 * H),
        "aw1g": np.ascontiguousarray(inputs["aw1g"], dtype=np.float32),
        "aw2g": np.ascontiguousarray(inputs["aw2g"], dtype=np.float32),
        "avg": np.ascontiguousarray(inputs["avg"], dtype=np.float32),
        "aw1c": np.ascontiguousarray(inputs["aw1c"], dtype=np.float32),
        "aw2c": np.ascontiguousarray(inputs["aw2c"], dtype=np.float32),
        "avc": np.ascontiguousarray(inputs["avc"], dtype=np.float32),
        "whw": np.ascontiguousarray(inputs["Wh"], dtype=np.float32),
        "bhT": np.ascontiguousarray(inputs["bh"], dtype=np.float32).reshape(H, 1),
        "wfw": np.ascontiguousarray(inputs["Wf"], dtype=np.float32),
        "bfv": np.ascontiguousarray(inputs["bf"], dtype=np.float32).reshape(1, 1),
    }
    enc_outs = np.asarray(inputs["enc_outs"], np.float32)
    enc_ins = np.asarray(inputs["enc_ins"], np.float32)
    Wg = np.asarray(inputs["Wg"], np.float32)
    bg = np.asarray(inputs["bg"], np.float32)

    in_maps = []
    for c in range(NC):
        sel = np.zeros((B, BL), np.float32)
        for lb in range(BL):
            sel[c * BL + lb, lb] = 1.0
        vsl = slice(c * VL, (c + 1) * VL)
        m = dict(common)
        m["sel"] = sel
        m["enc_outs_loc"] = np.ascontiguousarray(enc_outs[c * BL:(c + 1) * BL])
        m["wg_loc"] = np.ascontiguousarray(Wg[:, vsl])
        m["bg_loc"] = np.ascontiguousarray(bg[vsl]).reshape(1, VL)
        m["enc_ins_loc"] = np.ascontiguousarray(enc_ins[:, :, vsl])
        in_maps.append(m)
    return in_maps


def kernel(**inputs):
    nc = _get_program()
    in_maps = make_in_maps(inputs)
    res = run_bass_kernel_spmd(nc, in_maps, list(range(NC)))
    outs = res.results
    result = np.concatenate([outs[c]["result_loc"] for c in range(NC)], axis=1)
    return (
        result,
        outs[0]["hidden_att"],
        outs[0]["h1n"],
        outs[0]["c1n"],
        outs[0]["h2n"],
        outs[0]["c2n"],
    )


if __name__ == "__main__":
    nc = build_program()
    print("program built OK")


# revision 13
# speedup vs baseline: 1.0259x; 1.0259x over previous
"""Trainium2 Bass kernel for DecoderRNNCellJointCopy (B=16,T=600,V=20000,E=300,H=512).

Sharding across 8 NeuronCores:
  - LSTM stack: replicated (full batch, tiny work) on every core.
  - Bahdanau attentions: batch-parallel, 2 rows per core (row selection via a
    host-provided one-hot selector matrix so the program stays SPMD-uniform).
  - gen-prob Dense (Wg) + enc_ins copy-sum: vocab-parallel, V/8 = 2500 per core.
  - In-kernel AllGather shares per-row (exp copy-scores, hidden_att, S_c, switch);
    an AllReduce produces the global softmax denominator of the gen distribution.

Heavy matmuls run in float32r (full-rate single-pass fp32 mode, N>=256); small-N
matmuls stay plain fp32.
"""

import os

import ml_dtypes
import numpy as np

import concourse.bass as bass
import concourse.bacc as bacc
import concourse.mybir as mybir
import concourse.tile as tile
from concourse.bass_utils import run_bass_kernel_spmd
from concourse.masks import make_identity

F32 = mybir.dt.float32
F32R = mybir.dt.float32r
BF16 = mybir.dt.bfloat16
AF = mybir.ActivationFunctionType

B, T, V, E, H = 16, 600, 20000, 300, 512
NC = 8
BL = B // NC          # batch rows per core (2)
VL = V // NC          # vocab slice per core (2500)
KE = E + H            # 812
KEP = 896             # padded to 7*128
NKE = KEP // 128      # 7
NH = H // 128         # 4
TCH = [128, 128, 128, 128, 88]   # T=600 partition chunks
NTC = len(TCH)
VCW = 500             # psum-bank-sized vocab chunk
NVC = VL // VCW       # 5
CB = 640 + H + 2      # per-row gather payload: exp scores (padded 640) + hidden + S_c + switch


def ts(i, w):
    return bass.ts(i, w)


def build_program():
    nc = bacc.Bacc("TRN2", target_bir_lowering=False, debug=False, num_devices=NC)

    def inp(name, shape):
        return nc.dram_tensor(name, list(shape), F32, kind="ExternalInput").ap()

    def outp(name, shape):
        return nc.dram_tensor(name, list(shape), F32, kind="ExternalOutput").ap()

    embT = inp("embT", (KEP, B))
    h1T = inp("h1T", (H, B))
    h2T = inp("h2T", (H, B))
    c1_in = inp("c1_in", (B, H))
    c2_in = inp("c2_in", (B, H))
    k1p = inp("k1p", (KEP, 4 * H))
    r1w = inp("r1w", (H, 4 * H))
    k2w = inp("k2w", (H, 4 * H))
    r2w = inp("r2w", (H, 4 * H))
    b1v = inp("b1v", (1, 4 * H))
    b2v = inp("b2v", (1, 4 * H))
    aw1g = inp("aw1g", (H, H))
    aw2g = inp("aw2g", (H, H))
    avg = inp("avg", (H, 1))
    aw1c = inp("aw1c", (H, H))
    aw2c = inp("aw2c", (H, H))
    avc = inp("avc", (H, 1))
    whw = inp("whw", (2 * H, H))
    bhT = inp("bhT", (H, 1))
    wfw = inp("wfw", (H, 1))
    bfv = inp("bfv", (1, 1))
    sel = inp("sel", (B, BL))
    enc_outs_loc = inp("enc_outs_loc", (BL, T, H))
    wg_loc = inp("wg_loc", (H, VL))
    bg_loc = inp("bg_loc", (1, VL))
    enc_ins_loc = nc.dram_tensor("enc_ins_loc", [B, T, VL], BF16,
                                 kind="ExternalInput").ap()

    out_result = outp("result_loc", (B, VL))
    out_hidden = outp("hidden_att", (B, H))
    out_h1n = outp("h1n", (B, H))
    out_c1n = outp("c1n", (B, H))
    out_h2n = outp("h2n", (B, H))
    out_c2n = outp("c2n", (B, H))

    with tile.TileContext(nc) as tc:
        _body(nc, tc, locals())
    nc.compile()
    return nc


def _body(nc, tc, io):
    embT, h1T, h2T = io["embT"], io["h1T"], io["h2T"]
    c1_in, c2_in = io["c1_in"], io["c2_in"]
    k1p, r1w, k2w, r2w, b1v, b2v = (
        io["k1p"], io["r1w"], io["k2w"], io["r2w"], io["b1v"], io["b2v"])
    aw1g, aw2g, avg = io["aw1g"], io["aw2g"], io["avg"]
    aw1c, aw2c, avc = io["aw1c"], io["aw2c"], io["avc"]
    whw, bhT, wfw, bfv, sel = io["whw"], io["bhT"], io["wfw"], io["bfv"], io["sel"]
    enc_outs_loc, wg_loc, bg_loc, enc_ins_loc = (
        io["enc_outs_loc"], io["wg_loc"], io["bg_loc"], io["enc_ins_loc"])
    out_result, out_hidden = io["out_result"], io["out_hidden"]
    out_h1n, out_c1n, out_h2n, out_c2n = (
        io["out_h1n"], io["out_c1n"], io["out_h2n"], io["out_c2n"])

    with (
        tc.tile_pool(name="pers", bufs=1) as pers,
        tc.tile_pool(name="enc", bufs=6) as encp,
        tc.tile_pool(name="dram", bufs=1, space="DRAM") as dram,
    ):
        # ---------- persistent small tiles ----------
        ones = pers.tile([1, 128], F32)
        nc.gpsimd.memset(ones[:], 1.0)
        ones_r = pers.tile([1, 128], F32R)
        nc.vector.tensor_copy(ones_r[:], ones[:])
        onesC = pers.tile([128, 1], F32)
        nc.gpsimd.memset(onesC[:], 1.0)
        ident = pers.tile([128, 128], F32)
        make_identity(nc, ident[:])

        embT_sb = pers.tile([128, NKE, B], F32R)
        nc.sync.dma_start(embT_sb[:], embT.rearrange("(c p) n -> p c n", p=128).bitcast(F32R))
        h1T_sb = pers.tile([128, NH, B], F32R)
        nc.sync.dma_start(h1T_sb[:], h1T.rearrange("(c p) n -> p c n", p=128).bitcast(F32R))
        h2T_sb = pers.tile([128, NH, B], F32R)
        nc.sync.dma_start(h2T_sb[:], h2T.rearrange("(c p) n -> p c n", p=128).bitcast(F32R))
        c1_sb = pers.tile([B, H], F32)
        nc.sync.dma_start(c1_sb[:], c1_in)
        c2_sb = pers.tile([B, H], F32)
        nc.sync.dma_start(c2_sb[:], c2_in)
        b1_sb = pers.tile([1, 4 * H], F32R)
        nc.sync.dma_start(b1_sb[:], b1v.bitcast(F32R))
        b2_sb = pers.tile([1, 4 * H], F32R)
        nc.sync.dma_start(b2_sb[:], b2v.bitcast(F32R))
        sel_sb = pers.tile([B, BL], F32)
        nc.sync.dma_start(sel_sb[:], sel)

        # attention weights up-front (small, off the critical path later)
        def _wload(apin, c, n, dt=F32):
            t = pers.tile([128, c, n], dt, tag=f"w_{apin.tensor.name}")
            src = apin.rearrange("(c p) n -> p c n", p=128)
            if dt == F32R:
                src = src.bitcast(F32R)
            nc.sync.dma_start(t[:], src)
            return t

        aw1g_sb = _wload(aw1g, NH, H)
        aw2g_sb = _wload(aw2g, NH, H, F32R)
        aw1c_sb = _wload(aw1c, NH, H)
        aw2c_sb = _wload(aw2c, NH, H, F32R)
        wh_sb = _wload(whw, 2 * NH, H)
        avg_sb = pers.tile([128, NH], F32)
        nc.sync.dma_start(avg_sb[:], avg.rearrange("(c p) 1 -> p c", p=128))
        avc_sb = pers.tile([128, NH], F32)
        nc.sync.dma_start(avc_sb[:], avc.rearrange("(c p) 1 -> p c", p=128))
        wf_sb = pers.tile([128, NH], F32)
        nc.sync.dma_start(wf_sb[:], wfw.rearrange("(c p) 1 -> p c", p=128))
        bhT_sb = pers.tile([128, NH], F32)
        nc.sync.dma_start(bhT_sb[:], bhT.rearrange("(c p) 1 -> p c", p=128))
        bf_sb = pers.tile([1, 1], F32)
        nc.sync.dma_start(bf_sb[:], bfv)

        # persistent state tiles
        h1n_sb = pers.tile([B, H], F32)
        c1n_sb = pers.tile([B, H], F32)
        h2n_sb = pers.tile([B, H], F32)
        c2n_sb = pers.tile([B, H], F32)
        h1nT_sb = pers.tile([128, NH, B], F32R)
        h2nTl_sb = pers.tile([128, NH, BL], F32)
        qg_sb = pers.tile([128, NH, BL], F32)
        qc_sb = pers.tile([128, NH, BL], F32)
        escg_sb = pers.tile([128, BL, NTC], F32)
        escc_sb = pers.tile([128, BL, NTC], F32)
        catT_sb = pers.tile([128, 2 * NH, BL], F32)
        hattT_sb = pers.tile([128, NH, BL], F32)
        switch_sb = pers.tile([BL, 1], F32)
        sc_sb = pers.tile([1, BL], F32)      # raw copy-attention denominators
        rsg_sb = pers.tile([1, BL], F32)     # 1/S of gen-attention softmax
        rsbc_sb = pers.tile([128, BL], F32)  # broadcast of rsg over partitions
        # consumer-side (all-batch) tiles
        escT_sb = pers.tile([128, NTC, B], BF16)
        hattT_all = pers.tile([128, NH, B], F32R)
        swall_sb = pers.tile([B, 1], F32)
        scall_sb = pers.tile([B, 1], F32)
        sgpart_sb = pers.tile([B, NVC], F32)
        sgsum_sb = pers.tile([B, 1], F32)
        recipg_sb = pers.tile([B, 1], F32)
        recipc_sb = pers.tile([B, 1], F32)
        amix_sb = pers.tile([B, 1], F32)
        cmix_sb = pers.tile([B, 1], F32)

        # DRAM bounce buffers for collectives
        cbuf = dram.tile([BL, CB], F32)
        gath = dram.tile([B, CB], F32)
        sgin = dram.tile([B], F32)
        sgout = dram.tile([B], F32)

        # ---------- enc_ins stream (starts immediately; gated only by pool slots) ----------
        # allocated lazily in the copy phase below via encp.tile(tag="enc")

        # ---------- phase A: LSTM stack ----------
        with (
            tc.tile_pool(name="w1", bufs=1) as w1p,
            tc.tile_pool(name="psA", bufs=4, space="PSUM") as psA,
            tc.tile_pool(name="psTPa", bufs=2, space="PSUM") as psTPa,
            tc.tile_pool(name="gts", bufs=1) as gts,
        ):
            k1_sb = w1p.tile([128, NKE, 4 * H], F32R)
            nc.sync.dma_start(k1_sb[:], k1p.rearrange("(c p) n -> p c n", p=128).bitcast(F32R))
            r1_sb = w1p.tile([128, NH, 4 * H], F32R)
            nc.sync.dma_start(r1_sb[:], r1w.rearrange("(c p) n -> p c n", p=128).bitcast(F32R))

            def z_gates(xT_sb, nx, kw_sb, hT_sb, rw_sb, bias_sb, c_sb, hn_sb, cn_sb):
                zps = []
                for j in range(4):
                    ps = psA.tile([B, H], F32, tag="z")
                    for kc in range(nx):
                        nc.tensor.matmul(ps[:], xT_sb[:, kc, :],
                                         kw_sb[:, kc, ts(j, H)],
                                         start=(kc == 0), stop=False)
                    for kc in range(NH):
                        nc.tensor.matmul(ps[:], hT_sb[:, kc, :],
                                         rw_sb[:, kc, ts(j, H)],
                                         start=False, stop=False)
                    nc.tensor.matmul(ps[:], ones_r[0:1, 0:B],
                                     bias_sb[0:1, ts(j, H)],
                                     start=False, stop=True)
                    zps.append(ps)
                i_s = gts.tile([B, H], F32, tag="i_s")
                f_s = gts.tile([B, H], F32, tag="f_s")
                g_t = gts.tile([B, H], F32, tag="g_t")
                o_s = gts.tile([B, H], F32, tag="o_s")
                nc.scalar.activation(i_s[:], zps[0][:], AF.Sigmoid)
                nc.scalar.activation(f_s[:], zps[1][:], AF.Sigmoid)
                nc.scalar.activation(g_t[:], zps[2][:], AF.Tanh)
                nc.scalar.activation(o_s[:], zps[3][:], AF.Sigmoid)
                nc.vector.tensor_mul(f_s[:], f_s[:], c_sb[:])
                nc.vector.tensor_mul(i_s[:], i_s[:], g_t[:])
                nc.vector.tensor_add(cn_sb[:], f_s[:], i_s[:])
                nc.scalar.activation(g_t[:], cn_sb[:], AF.Tanh)
                nc.vector.tensor_mul(hn_sb[:], o_s[:], g_t[:])

            z_gates(embT_sb, NKE, k1_sb, h1T_sb, r1_sb, b1_sb, c1_sb, h1n_sb, c1n_sb)
            # transpose h1n -> [H, B] chunks for LSTM2 input
            for m in range(NH):
                pst = psTPa.tile([128, B], F32, tag="tpa")
                nc.tensor.transpose(pst[:], h1n_sb[:, ts(m, 128)], ident[0:B, 0:B])
                nc.scalar.activation(h1nT_sb[:, m, :], pst[:], AF.Copy)

        with (
            tc.tile_pool(name="w2", bufs=1) as w2p,
            tc.tile_pool(name="psA2", bufs=4, space="PSUM") as psA2,
            tc.tile_pool(name="gts2", bufs=1) as gts2,
        ):
            k2_sb = w2p.tile([128, NH, 4 * H], F32R)
            nc.sync.dma_start(k2_sb[:], k2w.rearrange("(c p) n -> p c n", p=128).bitcast(F32R))
            r2_sb = w2p.tile([128, NH, 4 * H], F32R)
            nc.sync.dma_start(r2_sb[:], r2w.rearrange("(c p) n -> p c n", p=128).bitcast(F32R))
            zps = []
            for j in range(4):
                ps = psA2.tile([B, H], F32, tag="z2")
                for kc in range(NH):
                    nc.tensor.matmul(ps[:], h1nT_sb[:, kc, :],
                                     k2_sb[:, kc, ts(j, H)],
                                     start=(kc == 0), stop=False)
                for kc in range(NH):
                    nc.tensor.matmul(ps[:], h2T_sb[:, kc, :],
                                     r2_sb[:, kc, ts(j, H)],
                                     start=False, stop=False)
                nc.tensor.matmul(ps[:], ones_r[0:1, 0:B],
                                 b2_sb[0:1, ts(j, H)],
                                 start=False, stop=True)
                zps.append(ps)
            i_s = gts2.tile([B, H], F32, tag="i_s2")
            f_s = gts2.tile([B, H], F32, tag="f_s2")
            g_t = gts2.tile([B, H], F32, tag="g_t2")
            o_s = gts2.tile([B, H], F32, tag="o_s2")
            nc.scalar.activation(i_s[:], zps[0][:], AF.Sigmoid)
            nc.scalar.activation(f_s[:], zps[1][:], AF.Sigmoid)
            nc.scalar.activation(g_t[:], zps[2][:], AF.Tanh)
            nc.scalar.activation(o_s[:], zps[3][:], AF.Sigmoid)
            nc.vector.tensor_mul(f_s[:], f_s[:], c2_sb[:])
            nc.vector.tensor_mul(i_s[:], i_s[:], g_t[:])
            nc.vector.tensor_add(c2n_sb[:], f_s[:], i_s[:])
            nc.scalar.activation(g_t[:], c2n_sb[:], AF.Tanh)
            nc.vector.tensor_mul(h2n_sb[:], o_s[:], g_t[:])

        # state outputs ready
        nc.gpsimd.dma_start(out_h1n, h1n_sb[:])
        nc.gpsimd.dma_start(out_c1n, c1n_sb[:])
        nc.gpsimd.dma_start(out_h2n, h2n_sb[:])
        nc.gpsimd.dma_start(out_c2n, c2n_sb[:])

        # ---------- phase B: attentions (local rows) ----------
        with (
            tc.tile_pool(name="attn", bufs=1) as attp,
            tc.tile_pool(name="tv", bufs=4) as tvp,
            tc.tile_pool(name="psS1", bufs=2, space="PSUM") as psS1,
            tc.tile_pool(name="psSC", bufs=2, space="PSUM") as psSC,
            tc.tile_pool(name="psM", bufs=3, space="PSUM") as psM,
        ):
            X_sb = attp.tile([128, BL, NTC, H], F32)
            for lb in range(BL):
                for tci, tw in enumerate(TCH):
                    nc.sync.dma_start(X_sb[0:tw, lb, tci, :],
                                      enc_outs_loc[lb, tci * 128:tci * 128 + tw, :])
            # local h2n columns via selector matmul: h2nT_loc = h2n^T @ sel
            for m in range(NH):
                psq = psM.tile([128, BL], F32, tag="m")
                nc.tensor.matmul(psq[:], h2n_sb[:, ts(m, 128)], sel_sb[:],
                                 start=True, stop=True)
                nc.scalar.activation(h2nTl_sb[:, m, :], psq[:], AF.Copy)
            # XT chunks
            XT_sb = attp.tile([128, BL, NH, T], F32R)
            for lb in range(BL):
                for tci, tw in enumerate(TCH):
                    for m in range(NH):
                        pst = psM.tile([128, 128], F32, tag="m")
                        nc.tensor.transpose(pst[0:128, 0:tw],
                                            X_sb[0:tw, lb, tci, ts(m, 128)],
                                            ident[0:tw, 0:tw])
                        nc.scalar.activation(
                            XT_sb[:, lb, m, tci * 128:tci * 128 + tw],
                            pst[0:128, 0:tw], AF.Copy)

            for w1_sb, w2_sb, av_sb, q_sb, esc_sb, is_gen in (
                (aw1g_sb, aw2g_sb, avg_sb, qg_sb, escg_sb, True),
                (aw1c_sb, aw2c_sb, avc_sb, qc_sb, escc_sb, False),
            ):
                # query projection qT = w1^T @ h2nT_loc
                for m in range(NH):
                    psq = psM.tile([128, BL], F32, tag="m")
                    for kc in range(NH):
                        nc.tensor.matmul(psq[:], w1_sb[:, kc, ts(m, 128)],
                                         h2nTl_sb[:, kc, :],
                                         start=(kc == 0), stop=(kc == NH - 1))
                    nc.scalar.activation(q_sb[:, m, :], psq[:], AF.Copy)
                for lb in range(BL):
                    nc.gpsimd.memset(esc_sb[:, lb, :], 0.0)
                    tvs = []
                    for m in range(NH):
                        tv = tvp.tile([128, T], F32, tag="tv")
                        for t0, twd in ((0, 512), (512, 88)):
                            ps1 = psS1.tile([128, 512], F32, tag="s1")
                            for kc in range(NH):
                                nc.tensor.matmul(
                                    ps1[:, 0:twd],
                                    w2_sb[:, kc, ts(m, 128)],
                                    XT_sb[:, lb, kc, t0:t0 + twd],
                                    start=(kc == 0), stop=(kc == NH - 1))
                            nc.scalar.activation(tv[:, t0:t0 + twd], ps1[:, 0:twd],
                                                 AF.Tanh, bias=q_sb[:, m, lb:lb + 1])
                        tvs.append(tv)
                    # scores (t-partitioned), exp
                    for tci, tw in enumerate(TCH):
                        pssc = psSC.tile([128, 1], F32, tag="sc")
                        for m in range(NH):
                            nc.tensor.matmul(
                                pssc[0:tw, 0:1],
                                tvs[m][:, tci * 128:tci * 128 + tw],
                                av_sb[:, m:m + 1],
                                start=(m == 0), stop=(m == NH - 1))
                        nc.scalar.activation(esc_sb[0:tw, lb, tci:tci + 1],
                                             pssc[0:tw, 0:1], AF.Exp)
                    # denominator
                    psS = psM.tile([128, 1], F32, tag="m")
                    for tci, tw in enumerate(TCH):
                        nc.tensor.matmul(psS[0:1, 0:1],
                                         esc_sb[0:tw, lb, tci:tci + 1],
                                         onesC[0:tw, 0:1],
                                         start=(tci == 0), stop=(tci == NTC - 1))
                    if is_gen:
                        nc.vector.reciprocal(rsg_sb[0:1, lb:lb + 1], psS[0:1, 0:1])
                        psb = psM.tile([128, 1], F32, tag="m")
                        nc.tensor.matmul(psb[:], ones[0:1, :],
                                         rsg_sb[0:1, lb:lb + 1],
                                         start=True, stop=True)
                        nc.scalar.activation(rsbc_sb[:, lb:lb + 1], psb[:], AF.Copy)
                        # context (normalized) into catT rows 0..511
                        for m in range(NH):
                            psc = psM.tile([128, 1], F32, tag="m")
                            for tci, tw in enumerate(TCH):
                                nc.tensor.matmul(psc[:],
                                                 X_sb[0:tw, lb, tci, ts(m, 128)],
                                                 esc_sb[0:tw, lb, tci:tci + 1],
                                                 start=(tci == 0),
                                                 stop=(tci == NTC - 1))
                            nc.vector.tensor_scalar_mul(catT_sb[:, m, lb:lb + 1],
                                                        psc[:], rsbc_sb[:, lb:lb + 1])
                    else:
                        nc.scalar.activation(sc_sb[0:1, lb:lb + 1], psS[0:1, 0:1],
                                             AF.Copy)

            # catT rows 512..1023 = h2n_loc
            for m in range(NH):
                nc.vector.tensor_copy(catT_sb[:, NH + m, :], h2nTl_sb[:, m, :])
            # hidden_att (transposed, local cols)
            for m in range(NH):
                psh = psM.tile([128, BL], F32, tag="m")
                for kc in range(2 * NH):
                    nc.tensor.matmul(psh[:], wh_sb[:, kc, ts(m, 128)],
                                     catT_sb[:, kc, :],
                                     start=(kc == 0), stop=(kc == 2 * NH - 1))
                nc.scalar.activation(hattT_sb[:, m, :], psh[:], AF.Tanh,
                                     bias=bhT_sb[:, m:m + 1])
            # switch
            pss = psM.tile([128, 1], F32, tag="m")
            for m in range(NH):
                nc.tensor.matmul(pss[0:BL, 0:1], hattT_sb[:, m, :],
                                 wf_sb[:, m:m + 1],
                                 start=(m == 0), stop=False)
            nc.tensor.matmul(pss[0:BL, 0:1], ones[0:1, 0:BL], bf_sb[0:1, 0:1],
                             start=False, stop=True)
            nc.scalar.activation(switch_sb[:], pss[0:BL, 0:1], AF.Sigmoid)

            # contribution buffer writes
            for lb in range(BL):
                for tci in range(NTC):
                    nc.gpsimd.dma_start(cbuf[lb, tci * 128:(tci + 1) * 128],
                                        escc_sb[:, lb, tci])
                for m in range(NH):
                    nc.gpsimd.dma_start(
                        cbuf[lb, 640 + m * 128:640 + (m + 1) * 128],
                        hattT_sb[:, m, lb])
                nc.gpsimd.dma_start(cbuf[lb, 640 + H:640 + H + 1],
                                    sc_sb[0:1, lb])
                nc.gpsimd.dma_start(cbuf[lb, 640 + H + 1:640 + H + 2],
                                    switch_sb[lb, 0:1])

        # ---------- AllGather ----------
        if os.environ.get("K_NO_COLLECTIVE"):
            nc.gpsimd.dma_start(gath[0:BL, :], cbuf[:, :])
        else:
            nc.gpsimd.collective_compute(
                "AllGather", mybir.AluOpType.bypass,
                replica_groups=[list(range(NC))],
                ins=[cbuf.opt()], outs=[gath.opt()])

        # ---------- phase C: gen dense + copy-sum on vocab slice ----------
        with (
            tc.tile_pool(name="gen", bufs=1) as genp,
            tc.tile_pool(name="stg", bufs=1) as stgp,
            tc.tile_pool(name="psTPc", bufs=1, space="PSUM") as psTPc,
            tc.tile_pool(name="psG", bufs=2, space="PSUM") as psG,
            tc.tile_pool(name="psCp", bufs=5, space="PSUM") as psCp,
        ):
            wg_sb = genp.tile([128, NH, VL], F32R)
            nc.sync.dma_start(wg_sb[:], wg_loc.rearrange("(c p) n -> p c n", p=128).bitcast(F32R))
            bg_sb = genp.tile([1, VL], F32R)
            nc.sync.dma_start(bg_sb[:], bg_loc.bitcast(F32R))
            escore_rows = genp.tile([B, 640], F32)
            nc.gpsimd.dma_start(escore_rows[:], gath[:, 0:640])
            hidden_rows = genp.tile([B, H], F32)
            nc.gpsimd.dma_start(hidden_rows[:], gath[:, 640:640 + H])
            nc.gpsimd.dma_start(scall_sb[:], gath[:, 640 + H:640 + H + 1])
            nc.gpsimd.dma_start(swall_sb[:], gath[:, 640 + H + 1:640 + H + 2])
            nc.gpsimd.dma_start(out_hidden, gath[:, 640:640 + H])

            for tci in range(NTC):
                pst = psTPc.tile([128, B], F32, tag="tpc")
                nc.tensor.transpose(pst[:], escore_rows[:, ts(tci, 128)],
                                    ident[0:B, 0:B])
                nc.scalar.activation(escT_sb[:, tci, :], pst[:], AF.Copy)
            for m in range(NH):
                pst = psTPc.tile([128, B], F32, tag="tpc")
                nc.tensor.transpose(pst[:], hidden_rows[:, ts(m, 128)],
                                    ident[0:B, 0:B])
                nc.scalar.activation(hattT_all[:, m, :], pst[:], AF.Copy)

            genexp_sb = genp.tile([B, VL], F32)
            for vc in range(NVC):
                pg = psG.tile([B, 512], F32, tag="pg")
                for kc in range(NH):
                    nc.tensor.matmul(pg[:, 0:VCW], hattT_all[:, kc, :],
                                     wg_sb[:, kc, ts(vc, VCW)],
                                     start=(kc == 0), stop=False)
                nc.tensor.matmul(pg[:, 0:VCW], ones_r[0:1, 0:B],
                                 bg_sb[0:1, ts(vc, VCW)],
                                 start=False, stop=True)
                nc.scalar.activation(genexp_sb[:, ts(vc, VCW)], pg[:, 0:VCW],
                                     AF.Exp, accum_out=sgpart_sb[:, vc:vc + 1])
            nc.vector.tensor_reduce(sgsum_sb[:], sgpart_sb[:],
                                    axis=mybir.AxisListType.X,
                                    op=mybir.AluOpType.add)
            nc.gpsimd.dma_start(sgin[:], sgsum_sb[:])
            if os.environ.get("K_NO_COLLECTIVE"):
                nc.gpsimd.dma_start(sgout[:], sgin[:])
            else:
                nc.gpsimd.collective_compute(
                    "AllReduce", mybir.AluOpType.add,
                    replica_groups=[list(range(NC))],
                    ins=[sgin.opt()], outs=[sgout.opt()])
            nc.gpsimd.dma_start(sgsum_sb[:], sgout[:])
            nc.vector.reciprocal(recipg_sb[:], sgsum_sb[:])
            nc.vector.reciprocal(recipc_sb[:], scall_sb[:])
            nc.vector.tensor_scalar(amix_sb[:], swall_sb[:], -1.0, 1.0,
                                    op0=mybir.AluOpType.mult,
                                    op1=mybir.AluOpType.add)
            nc.vector.tensor_mul(amix_sb[:], amix_sb[:], recipg_sb[:])
            nc.vector.tensor_mul(cmix_sb[:], swall_sb[:], recipc_sb[:])

            # copy-sum over the enc_ins stream
            copyfull_sb = genp.tile([B, VL], F32)
            for g in range(B):
                pcs = [psCp.tile([1, 512], F32, tag="cp", name=f"cp{vc}")
                       for vc in range(NVC)]
                for tci, tw in enumerate(TCH):
                    et = encp.tile([128, VL], BF16, tag="enc")
                    nc.sync.dma_start(et[0:tw, :],
                                      enc_ins_loc[g, tci * 128:tci * 128 + tw, :])
                    for vc in range(NVC):
                        nc.tensor.matmul(pcs[vc][0:1, 0:VCW],
                                         escT_sb[0:tw, tci, g:g + 1],
                                         et[0:tw, ts(vc, VCW)],
                                         start=(tci == 0), stop=(tci == NTC - 1))
                stage = stgp.tile([1, VL], F32, tag="stage")
                for vc in range(NVC):
                    nc.scalar.activation(stage[0:1, ts(vc, VCW)],
                                         pcs[vc][0:1, 0:VCW], AF.Copy)
                nc.gpsimd.dma_start(copyfull_sb[g:g + 1, :], stage[0:1, :])

            # final mix and output
            fin_sb = genp.tile([B, VL], F32)
            for vc in range(NVC):
                sl = ts(vc, VCW)
                nc.vector.tensor_scalar_mul(fin_sb[:, sl], genexp_sb[:, sl],
                                            amix_sb[:])
                nc.vector.tensor_scalar_mul(copyfull_sb[:, sl], copyfull_sb[:, sl],
                                            cmix_sb[:])
                nc.vector.tensor_add(fin_sb[:, sl], fin_sb[:, sl],
                                     copyfull_sb[:, sl])
            nc.sync.dma_start(out_result, fin_sb[:])


_PROGRAM = None


def _get_program():
    global _PROGRAM
    if _PROGRAM is None:
        _PROGRAM = build_program()
    return _PROGRAM


def make_in_maps(inputs):
    x = np.asarray(inputs["x"])
    emb_table = np.asarray(inputs["emb_table"], np.float32)
    emb = emb_table[x[:, 0]]                                   # [B, E]
    emb_att = np.concatenate(
        [emb, np.asarray(inputs["last_hidden_attn"], np.float32)], axis=1)
    embT = np.zeros((KEP, B), np.float32)
    embT[:KE, :] = emb_att.T
    k1 = np.asarray(inputs["k1"], np.float32)
    k1p = np.zeros((KEP, 4 * H), np.float32)
    k1p[:KE, :] = k1

    common = {
        "embT": np.ascontiguousarray(embT),
        "h1T": np.ascontiguousarray(np.asarray(inputs["h1"], np.float32).T),
        "h2T": np.ascontiguousarray(np.asarray(inputs["h2"], np.float32).T),
        "c1_in": np.ascontiguousarray(inputs["c1"], dtype=np.float32),
        "c2_in": np.ascontiguousarray(inputs["c2"], dtype=np.float32),
        "k1p": k1p,
        "r1w": np.ascontiguousarray(inputs["r1"], dtype=np.float32),
        "k2w": np.ascontiguousarray(inputs["k2"], dtype=np.float32),
        "r2w": np.ascontiguousarray(inputs["r2"], dtype=np.float32),
        "b1v": np.ascontiguousarray(inputs["b1"], dtype=np.float32).reshape(1, 4 * H),
        "b2v": np.ascontiguousarray(inputs["b2"], dtype=np.float32).reshape(1, 4 * H),
        "aw1g": np.ascontiguousarray(inputs["aw1g"], dtype=np.float32),
        "aw2g": np.ascontiguousarray(inputs["aw2g"], dtype=np.float32),
        "avg": np.ascontiguousarray(inputs["avg"], dtype=np.float32),
        "aw1c": np.ascontiguousarray(inputs["aw1c"], dtype=np.float32),
        "aw2c": np.ascontiguousarray(inputs["aw2c"], dtype=np.float32),
        "avc": np.ascontiguousarray(inputs["avc"], dtype=np.float32),
        "whw": np.ascontiguousarray(inputs["Wh"], dtype=np.float32),
        "bhT": np.ascontiguousarray(inputs["bh"], dtype=np.float32).reshape(H, 1),
        "wfw": np.ascontiguousarray(inputs["Wf"], dtype=np.float32),
        "bfv": np.ascontiguousarray(inputs["bf"], dtype=np.float32).reshape(1, 1),
    }
    enc_outs = np.asarray(inputs["enc_outs"], np.float32)
    enc_ins = np.asarray(inputs["enc_ins"], np.float32)
    Wg = np.asarray(inputs["Wg"], np.float32)
    bg = np.asarray(inputs["bg"], np.float32)

    in_maps = []
    for c in range(NC):
        sel = np.zeros((B, BL), np.float32)
        for lb in range(BL):
            sel[c * BL + lb, lb] = 1.0
        vsl = slice(c * VL, (c + 1) * VL)
        m = dict(common)
        m["sel"] = sel
        m["enc_outs_loc"] = np.ascontiguousarray(enc_outs[c * BL:(c + 1) * BL])
        m["wg_loc"] = np.ascontiguousarray(Wg[:, vsl])
        m["bg_loc"] = np.ascontiguousarray(bg[vsl]).reshape(1, VL)
        m["enc_ins_loc"] = np.ascontiguousarray(
            enc_ins[:, :, vsl]).astype(ml_dtypes.bfloat16)
        in_maps.append(m)
    return in_maps


def kernel(**inputs):
    nc = _get_program()
    in_maps = make_in_maps(inputs)
    res = run_bass_kernel_spmd(nc, in_maps, list(range(NC)))
    outs = res.results
    result = np.concatenate([outs[c]["result_loc"] for c in range(NC)], axis=1)
    return (
        result,
        outs[0]["hidden_att"],
        outs[0]["h1n"],
        outs[0]["c1n"],
        outs[0]["h2n"],
        outs[0]["c2n"],
    )


if __name__ == "__main__":
    nc = build_program()
    print("program built OK")
